# revision 7
# baseline (speedup 1.0000x reference)
# Conv2d 3x3 VALID stride-1 as implicit GEMM on 8 TRN2 NeuronCores.
#
# Problem: x[32,128,56,56] f32, weight[256,128,3,3] f32, bias[256] f32
#          -> out[32,256,54,54] f32
#
# Sharding: data-parallel over batch — 4 images per core, weight replicated.
# Per-core kernel: for each (image, oc-half, row-tile) accumulate the 9
# kernel-position matmuls into PSUM (contraction dim = 128 input channels on
# the partition axis), evict through ScalarE (bias add) and DMA out.
# Compute dtype: fp16 (PE runs fp16 at 1 cycle/row vs 4 for fp32; PSUM
# accumulation is fp32; measured rel err vs f32 reference ~3e-4).

import numpy as np

import concourse.bass as bass
import concourse.tile as tile
from concourse import bacc, mybir
from concourse.bass_utils import run_bass_kernel_spmd

N_CORES = 8
IMGS = 4          # images per core
IC = 128
OC = 256
H = W = 56
OH = OW = 54
KH = KW = 3
ROWTILE = 18      # output rows per PSUM tile (2 chunks of 9 rows)
CHUNK_ROWS = 9    # output rows per matmul (N = 9*54 = 486 <= 512)
NCHUNK = ROWTILE // CHUNK_ROWS
NTILE = OH // ROWTILE

FP16 = mybir.dt.float16
F32 = mybir.dt.float32


def build_conv_bass(repeat=1, num_devices=N_CORES):
    nc = bacc.Bacc(
        "TRN2",
        target_bir_lowering=False,
        debug=False,
        num_devices=num_devices,
    )
    x_ext = nc.dram_tensor("x", [IMGS, IC, H, W], FP16, kind="ExternalInput")
    wt_ext = nc.dram_tensor("wt", [IC, KH, KW, OC], FP16, kind="ExternalInput")
    b_ext = nc.dram_tensor("bias", [128, 2], F32, kind="ExternalInput")
    out_ext = nc.dram_tensor("out", [IMGS, OC, OH, OW], F32, kind="ExternalOutput")

    with tile.TileContext(nc) as tc:
        with (
            tc.tile_pool(name="consts", bufs=1) as cpool,
            tc.tile_pool(name="xin", bufs=IMGS) as xpool,
            tc.tile_pool(name="psum", bufs=4, space="PSUM") as ppool,
            tc.tile_pool(name="outs", bufs=3) as opool,
        ):
            w_sb = cpool.tile([IC, KH, KW, OC], FP16)
            nc.sync.dma_start(w_sb[:], wt_ext[:])
            b_sb = cpool.tile([128, 2], F32)
            nc.sync.dma_start(b_sb[:], b_ext[:])

            x_tiles = []
            for img in range(IMGS):
                xt = xpool.tile([IC, H, W], FP16, tag=f"x{img}")
                nc.sync.dma_start(xt[:], x_ext[img])
                x_tiles.append(xt)

            for _rep in range(repeat):
              for img in range(IMGS):
                for och in range(2):
                    for t in range(NTILE):
                        ps = ppool.tile([128, NCHUNK, 512], F32)
                        for kh in range(KH):
                            for kw in range(KW):
                                lhsT = w_sb[:, kh, kw, och * 128:(och + 1) * 128]
                                for c in range(NCHUNK):
                                    r0 = t * ROWTILE + c * CHUNK_ROWS
                                    rhs = x_tiles[img][
                                        :, r0 + kh:r0 + kh + CHUNK_ROWS, kw:kw + OW
                                    ]
                                    nc.tensor.matmul(
                                        ps[:, c, 0:CHUNK_ROWS * OW],
                                        lhsT,
                                        rhs,
                                        start=(kh == 0 and kw == 0),
                                        stop=(kh == KH - 1 and kw == KW - 1),
                                    )
                        ob = opool.tile([128, NCHUNK, CHUNK_ROWS * OW], F32)
                        for c in range(NCHUNK):
                            nc.scalar.activation(
                                ob[:, c],
                                ps[:, c, 0:CHUNK_ROWS * OW],
                                mybir.ActivationFunctionType.Identity,
                                bias=b_sb[:, och:och + 1],
                            )
                        nc.sync.dma_start(
                            out_ext[
                                img,
                                och * 128:(och + 1) * 128,
                                t * ROWTILE:(t + 1) * ROWTILE,
                                :,
                            ],
                            ob[:],
                        )
    nc.compile()
    return nc


_CACHE = {}


def _get_nc(repeat=1):
    if repeat not in _CACHE:
        _CACHE[repeat] = build_conv_bass(repeat=repeat)
    return _CACHE[repeat]


def kernel(x, weight, bias, _want_results_obj=False, _repeat=1, **run_kwargs):
    assert x.shape == (32, IC, H, W)
    x16 = np.ascontiguousarray(x.astype(np.float16))
    # weight [oc, ic, kh, kw] -> lhsT layout [ic, kh, kw, oc]
    wt = np.ascontiguousarray(weight.astype(np.float16).transpose(1, 2, 3, 0))
    b2 = np.ascontiguousarray(
        bias.astype(np.float32).reshape(2, 128).T
    )  # [128, 2]: b2[p, h] = bias[h*128+p]

    nc = _get_nc(_repeat)
    in_maps = [
        {"x": x16[i * IMGS:(i + 1) * IMGS], "wt": wt, "bias": b2}
        for i in range(N_CORES)
    ]
    res = run_bass_kernel_spmd(nc, in_maps, core_ids=list(range(N_CORES)), **run_kwargs)
    out = np.concatenate([res.results[i]["out"] for i in range(N_CORES)], axis=0)
    if _want_results_obj:
        return out, res
    return out
